# revision 12
# baseline (speedup 1.0000x reference)
"""Conv3d(32->64, k=3, pad=1) + BatchNorm(training) + LeakyReLU(0.2) on
(2, 32, 96, 96, 35), distributed over 8 TRN2 NeuronCores.

v2 strategy (from v1 trace analysis: conv 195us, collective gap 66us,
pass-2 100us -> 383us total):
  - Shard H (96 = 8 x 12 rows per core); halo + spatial zero-padding
    materialized host-side into xs (2,32,14,98,37) bf16.
  - Conv as implicit GEMM bf16, K = 96 = C_in(32) x kd(3) via 3 d-shifted
    slab partition groups; 9 (kh,kw) taps accumulate in PSUM.  Two w-tiles
    run concurrently as a PE column pair (psum[0:64] / psum[64:128]).
  - v2: tap-outer loop over 4 psum tiles (weight reused across 4 w-tile
    pairs), N trimmed 444->420 via 2-dim rhs AP (skips the d-pad columns).
  - BN stats: per-tile sums via ACT accum_out on the eviction copy; sumsq
    via DVE tensor_tensor_reduce (skips bn_aggr on the critical path).
    Cross-core reduction by a tiny AllReduce.  A/B column half fold +
    landing on all 128 partitions in one f32 matmul (replaces two
    partition-move DMAs and the 64->128 broadcast).
  - Pass 2: Prelu split ACT (9 iters) / DVE 3-op (3 iters); output stores
    round-robin over 3 DMA rings (SP, PE, DVE).
"""

import numpy as np
import ml_dtypes

import concourse.bacc as bacc
import concourse.bass as bass
import concourse.tile as tile
from concourse import mybir
from concourse.bass_utils import run_bass_kernel_spmd

N_CORES = 8
B, C_IN, C_OUT = 2, 32, 64
H, W, D = 96, 96, 35
HS = H // N_CORES          # 12 output rows per core
HR = HS + 2                # 14 input rows (halo)
WP, DP = W + 2, D + 2      # padded W / padded D for the host tensor
RW = D + 2                 # 37: slab row width per w-column (full padded D)
SLAB = WP * RW + 2 + 30    # slab row extent incl. group-shift + slack
WT = 12                    # w-tile width (8 uniform tiles)
NFULL = WT * RW            # 444 matmul free size (incl. d-pad columns)
EVF = WT * D               # 420 columns per tile
BLK = B * HS               # 24 (b,h) blocks per core
BLKCOLS = 4 * EVF          # 1680 conv-buffer columns per block per half
NTILE = BLK * 4            # 96 (block, ktile) tiles per core
CNT = float(BLK * 4 * EVF)          # 40320 elements per partition, local
N_TOT = float(B * H * W * D)        # 645120 elements per channel, global
EPS = 1e-5
NEG = 0.2

DVE_P2 = (3, 7, 11)        # pass-2 iterations handled by the vector engine

F32 = mybir.dt.float32
BF16 = mybir.dt.bfloat16
NP_BF16 = ml_dtypes.bfloat16

_CACHE = {}


def _build():
    nc = bacc.Bacc("TRN2", target_bir_lowering=False, debug=False,
                   num_devices=N_CORES)
    xs = nc.dram_tensor("xs", [B, C_IN, HR, WP, DP], BF16, kind="ExternalInput")
    wt = nc.dram_tensor("wt", [3, 3, 96, C_OUT], BF16, kind="ExternalInput")
    gm = nc.dram_tensor("gm", [128], F32, kind="ExternalInput")
    bt = nc.dram_tensor("bt", [128], F32, kind="ExternalInput")
    fw = nc.dram_tensor("fw", [128, 128], F32, kind="ExternalInput")
    ys = nc.dram_tensor("ys", [B, C_OUT, HS, W, D], F32, kind="ExternalOutput")

    xs_ap = xs.ap()
    ys_ap = ys.ap()

    from contextlib import ExitStack
    with tile.TileContext(nc) as tc:
        with tc.tile_pool(name="singles", bufs=1) as singles, \
             tc.tile_pool(name="dram", bufs=1, space="DRAM") as dramp:
            phase1 = ExitStack()
            slabp = phase1.enter_context(tc.tile_pool(name="slab", bufs=4))
            psump = phase1.enter_context(
                tc.tile_pool(name="psum", bufs=8, space="PSUM"))

            # ---- one-time loads ----
            wtile = singles.tile([96, 9, C_OUT], BF16)
            nc.sync.dma_start(
                out=wtile,
                in_=wt.ap().rearrange("kh kw p o -> p (kh kw) o"))
            gmt = singles.tile([128, 1], F32)
            nc.sync.dma_start(out=gmt, in_=gm.ap().rearrange("(p o) -> p o", o=1))
            btt = singles.tile([128, 1], F32)
            nc.sync.dma_start(out=btt, in_=bt.ap().rearrange("(p o) -> p o", o=1))
            foldw = singles.tile([128, 128], F32)
            nc.sync.dma_start(out=foldw, in_=fw.ap())

            cb = singles.tile([128, BLK * BLKCOLS], BF16)   # conv results
            st = singles.tile([128, NTILE * 6], F32)        # bn_stats records
            sq = singles.tile([128, 2], F32)                # local (sum, sumsq)

            epst = singles.tile([128, 1], F32)
            nc.vector.memset(epst, EPS)
            tblscr = singles.tile([128, 1], F32)
            # preload the ACT Sqrt table off the critical path
            nc.scalar.activation(out=tblscr, in_=epst,
                                 func=mybir.ActivationFunctionType.Sqrt,
                                 bias=epst)

            # ---- pass 1: conv + stats ----
            for b in range(B):
                groups = {}
                for h in range(HS):
                    for r in (h, h + 1, h + 2):
                        g = r // 2
                        if g not in groups:
                            gt = slabp.tile([96, 2, SLAB], BF16, tag="slab")
                            for j in range(3):
                                # group j holds the rows shifted by (2-j)
                                nc.sync.dma_start(
                                    out=gt[32 * j:32 * (j + 1), :,
                                           2 - j:2 - j + WP * RW],
                                    in_=xs_ap[b, :, 2 * g:2 * g + 2, :, :].rearrange(
                                        "p r w d -> p r (w d)"))
                            groups[g] = gt
                    blk = b * HS + h
                    ps4 = [psump.tile([128, NFULL], F32, tag="ps", name=f"ps{k}")
                           for k in range(4)]
                    for q in range(9):
                        kh, kw = divmod(q, 3)
                        r = h + kh
                        gt = groups[r // 2]
                        rs = r % 2
                        first, last = q == 0, q == 8
                        for k in range(4):
                            w0a = 2 * k * WT
                            w0b = (2 * k + 1) * WT
                            oa = (w0a + kw) * RW + 2
                            ob = (w0b + kw) * RW + 2
                            nc.tensor.matmul(
                                ps4[k][0:64, :],
                                lhsT=wtile[:, q, :],
                                rhs=gt[:, rs, oa:oa + NFULL],
                                start=first, stop=last)
                            nc.tensor.matmul(
                                ps4[k][64:128, :],
                                lhsT=wtile[:, q, :],
                                rhs=gt[:, rs, ob:ob + NFULL],
                                start=first, stop=last)
                    # evict + stats
                    for k in range(4):
                        col = blk * BLKCOLS + k * EVF
                        rec = (blk * 4 + k) * 6
                        pv = ps4[k].rearrange("p (w d) -> p w d", d=RW)[:, :, 0:D]
                        nc.scalar.copy(
                            out=cb[:, col:col + EVF].rearrange(
                                "p (w d) -> p w d", d=D),
                            in_=pv)
                        nc.vector.bn_stats(out=st[:, rec:rec + 6],
                                           in_=cb[:, col:col + EVF])

            phase1.close()

            # ---- local stats total + cross-core exchange ----
            mv = singles.tile([128, 2], F32)
            nc.vector.bn_aggr(out=mv, in_=st.rearrange("p (r s) -> p r s", s=6))
            tcv = singles.tile([128, 1], F32)
            nc.vector.tensor_scalar_mul(sq[:, 0:1], mv[:, 0:1], CNT)
            nc.vector.tensor_mul(tcv, mv[:, 0:1], mv[:, 0:1])
            nc.vector.tensor_add(tcv, tcv, mv[:, 1:2])
            nc.vector.tensor_scalar_mul(sq[:, 1:2], tcv, CNT)
            cc_in = dramp.tile([128, 2], F32)
            cc_out = dramp.tile([128, 2], F32)
            nc.sync.dma_start(out=cc_in[:, :], in_=sq)
            nc.gpsimd.collective_compute(
                "AllReduce", mybir.AluOpType.add,
                replica_groups=[list(range(N_CORES))],
                ins=[cc_in[:, :].opt()], outs=[cc_out[:, :].opt()])
            gl = singles.tile([128, 2], F32)
            nc.sync.dma_start(out=gl, in_=cc_out[:, :])

            # fold the two PE column halves (partitions p / p+64 hold the
            # same channel) and land the result on all 128 partitions
            psmid = phase1.enter_context(
                tc.tile_pool(name="psmid", bufs=1, space="PSUM"))
            pst = psmid.tile([128, 2], F32)
            nc.tensor.matmul(pst, lhsT=foldw, rhs=gl, start=True, stop=True)

            m_g = singles.tile([128, 1], F32)
            qn = singles.tile([128, 1], F32)
            var = singles.tile([128, 1], F32)
            sd = singles.tile([128, 1], F32)
            s_all = singles.tile([128, 1], F32)
            t_all = singles.tile([128, 1], F32)
            t1 = singles.tile([128, 1], F32)
            nc.vector.tensor_scalar_mul(m_g, pst[:, 0:1], 1.0 / N_TOT)
            nc.vector.tensor_scalar_mul(qn, pst[:, 1:2], 1.0 / N_TOT)
            nc.vector.tensor_mul(var, m_g, m_g)
            nc.vector.tensor_sub(var, qn, var)
            nc.scalar.activation(out=sd, in_=var,
                                 func=mybir.ActivationFunctionType.Sqrt,
                                 bias=epst)
            nc.vector.reciprocal(out=sd, in_=sd)
            nc.vector.tensor_mul(s_all, sd, gmt)    # s = gamma * rsqrt(var+eps)
            nc.vector.tensor_mul(t1, m_g, s_all)
            nc.vector.tensor_sub(t_all, btt, t1)    # t = beta - mean * s

            # ---- pass 2: normalize + LeakyReLU + writeback ----
            stgp = phase1.enter_context(tc.tile_pool(name="stg", bufs=6))
            zscr = singles.tile([128, 2 * BLKCOLS], F32)
            queues = (nc.sync, nc.scalar)
            c_step = HS * W * D  # ys channel stride
            for it in range(12):
                blk = 2 * it
                b_, h_ = divmod(blk, HS)
                stg = stgp.tile([128, 2 * BLKCOLS], F32, tag="stg")
                src = cb[:, blk * BLKCOLS:(blk + 2) * BLKCOLS]
                if it in DVE_P2:
                    # z = s*x + t ; y = max(z, 0.2*z)
                    nc.vector.tensor_scalar(
                        stg, src, s_all, t_all,
                        mybir.AluOpType.mult, mybir.AluOpType.add)
                    nc.vector.tensor_scalar_mul(zscr, stg, NEG)
                    nc.vector.tensor_tensor(
                        out=stg, in0=stg, in1=zscr, op=mybir.AluOpType.max)
                else:
                    nc.scalar.activation(
                        out=stg, in_=src,
                        func=mybir.ActivationFunctionType.Prelu,
                        bias=t_all, scale=s_all, alpha=NEG)
                base_off = ys_ap.offset + b_ * (C_OUT * c_step) + h_ * (W * D)
                # two consecutive h rows; w-tiles at w = 0,24,48,72 (A) / +12 (B)
                dst_a = bass.AP(
                    tensor=ys_ap.tensor, offset=base_off,
                    ap=[[c_step, C_OUT], [W * D, 2], [2 * WT * D, 4],
                        [D, WT], [1, D]])
                queues[0].dma_start(
                    out=dst_a,
                    in_=stg[0:64, :].rearrange(
                        "p (r t w d) -> p r t w d", r=2, t=4, d=D))
                dst_b = bass.AP(
                    tensor=ys_ap.tensor, offset=base_off + WT * D,
                    ap=[[c_step, C_OUT], [W * D, 2], [2 * WT * D, 4],
                        [D, WT], [1, D]])
                queues[1].dma_start(
                    out=dst_b,
                    in_=stg[64:128, :].rearrange(
                        "p (r t w d) -> p r t w d", r=2, t=4, d=D))

            phase1.close()
    nc.finalize()
    return nc


def _get_nc():
    if "nc" not in _CACHE:
        _CACHE["nc"] = _build()
    return _CACHE["nc"]


def _prep(x, w, gamma, beta):
    xpad = np.zeros((B, C_IN, H + 2, WP, DP), dtype=np.float32)
    xpad[:, :, 1:H + 1, 1:W + 1, 1:D + 1] = x
    wt = np.ascontiguousarray(
        np.asarray(w, dtype=np.float32).transpose(2, 3, 4, 1, 0).reshape(
            3, 3, 96, C_OUT)).astype(NP_BF16)
    gm = np.ascontiguousarray(
        np.tile(np.asarray(gamma, dtype=np.float32), 2))
    bt = np.ascontiguousarray(
        np.tile(np.asarray(beta, dtype=np.float32), 2))
    pp = np.arange(128)
    fw = (pp[:, None] % 64 == pp[None, :] % 64).astype(np.float32)
    in_maps = []
    for c in range(N_CORES):
        xsl = np.ascontiguousarray(
            xpad[:, :, c * HS:c * HS + HR, :, :]).astype(NP_BF16)
        in_maps.append({"xs": xsl, "wt": wt, "gm": gm, "bt": bt, "fw": fw})
    return in_maps


def kernel(x, w, b, gamma, beta):
    nc = _get_nc()
    in_maps = _prep(np.asarray(x, dtype=np.float32), w, gamma, beta)
    res = run_bass_kernel_spmd(nc, in_maps, core_ids=list(range(N_CORES)))
    out = np.concatenate([res.results[c]["ys"] for c in range(N_CORES)], axis=2)
    return out.astype(np.float32)
